# revision 28
# baseline (speedup 1.0000x reference)
"""MetaSage GNN kernel for 8 Trainium2 NeuronCores (Bass/Tile, SPMD).

Strategy (graph/edge parallel, dst-node sharded):
- Destination nodes sharded contiguously across 8 cores (products 12500/core,
  customers 6250/core). Edges bucketed by dst shard on host, sorted by dst,
  grouped into dst-tiles of 128 nodes, each tile's edge list padded to M
  chunks of 128 edges (M = global max, uniform -> single SPMD program).
- Per chunk: indirect-DMA gather of 128 source rows, one-hot [edge, dst]
  built on DVE (is_equal vs iota), segment-sum via PE matmul accumulating
  into PSUM [128 dst, 128 feat + 1 count col].
- mean = sum/max(cnt,1); SAGE linear in transposed orientation
  h = relu(Wl @ meanT + b + Wr @ xT) on PE; layer-1 aggregation shared
  between item and user encoders.
- AllGather (p, px) feeds layer-2 gathers; decoder linears folded into the
  node-level linears on host (z_cust/z_prod never materialized):
    ZC' = cx2 @ (W1L us_Wlin).T + (de_b1 + W1L us_blin + W1R it_blin)
    ZP' = p2 @ (W1R it_Wlin).T
    out[e] = w2 . relu(ZC'[row] + ZP'[col]) + de_b2
- Decoder: gather ZC'/ZP' rows, add+relu+mul+grouped-reduce on DVE.

Host<->device transfer optimizations (the wall-clock bottleneck under the
axon PJRT relay is per-array overhead, not bandwidth):
- ALL per-core inputs are packed into ONE f32 blob; int sections are
  bitcast to i32 on device, x features are shipped fp16 and converted
  on device (transposes for the self path are built on the PE).
- jax persistent compilation cache enabled so per-call XLA re-compiles
  (fresh jit inside run_bass_kernel_spmd) hit disk after the first call.
"""
import os
import numpy as np
from contextlib import ExitStack

import jax

for _k, _v in [("jax_compilation_cache_dir", os.path.expanduser("~/.jax_cache")),
               ("jax_persistent_cache_min_entry_size_bytes", -1),
               ("jax_persistent_cache_min_compile_time_secs", 0.0)]:
    try:
        jax.config.update(_k, _v)
    except Exception:
        pass

from concourse import bass, bacc, mybir
from concourse import bass_utils
from concourse.bass import ds
import concourse.tile as tile
from concourse.masks import make_identity

USE_FORI = os.environ.get("KFORI", "1") == "1"

P = 128
NCORES = 8
N_PROD, N_CUST = 100000, 50000
HID, OUT = 128, 64
E_LB = 400000
PS = N_PROD // NCORES          # 12500 product dsts per core
CS = N_CUST // NCORES          # 6250 customer dsts per core
PT = (PS + P - 1) // P         # 98 tiles
CT = (CS + P - 1) // P         # 49 tiles
PSP = PT * P                   # 12544 padded product shard
CSP = CT * P                   # 6272 padded customer shard
PFULL = NCORES * PSP           # 100352
CFULL = NCORES * CSP           # 50176
EC = E_LB // NCORES            # 50000 label edges per core
DEC = (EC + P - 1) // P        # 391 chunks
DECN = DEC * P                 # 50048
GD = 4                         # decoder chunk grouping
F32 = mybir.dt.float32
F16 = mybir.dt.float16
I32 = mybir.dt.int32

_cache = {}


def _blob_layout(M1, M2):
    off = {}
    o = 0
    for name, words in [
        ("xp32", PSP * HID), ("xc32", CSP * HID),
        ("doff_pp", P * PT * M1 // 4), ("doff_pc", P * CT * M2 // 4),
        ("wcat", 10 * HID * HID), ("WpT", HID * OUT), ("WcT", HID * OUT),
        ("bias8", P * 8), ("w2rep", P * OUT),
        ("idx_pp", P * PT * M1), ("idx_pc", P * CT * M2),
        ("dec_row", P * DEC // 2), ("dec_col", P * DEC),
    ]:
        off[name] = o
        o += words
    return off, o


def _bucket_edges(src, dst, S, T):
    """Bucket edges by dst shard, sort by local dst. -> per-core (srcs, ldst)."""
    src = np.asarray(src).astype(np.int64)
    dst = np.asarray(dst).astype(np.int64)
    out = []
    core = dst // S
    for c in range(NCORES):
        m = core == c
        s_c, ld = src[m], dst[m] - c * S
        o = np.argsort(ld, kind="stable")
        out.append((s_c[o], ld[o]))
    return out


def _edge_tiles(buckets, T):
    """-> M (global chunks/tile), per-core (idx[128,T*M] int32 raw-src, doff)."""
    M = 1
    infos = []
    for s_c, ld in buckets:
        tid = ld >> 7
        cnt = np.bincount(tid, minlength=T)
        M = max(M, int((cnt.max() + P - 1) // P))
        starts = np.concatenate([[0], np.cumsum(cnt)])
        k = np.arange(len(ld)) - starts[tid]
        infos.append((s_c, ld, tid, k))
    idxs, doffs = [], []
    for s_c, ld, tid, k in infos:
        col = tid * M + (k >> 7)
        row = k & 127
        idx = np.zeros((P, T * M), np.int32)
        doff = np.full((P, T * M), 255, np.uint8)
        idx[row, col] = s_c
        doff[row, col] = (ld - (tid << 7)).astype(np.uint8)
        idxs.append(idx)
        doffs.append(doff)
    return M, idxs, doffs


def _remap_prod(g):
    return ((g // PS) * PSP + g % PS).astype(np.int32)


def _remap_cust(g):
    return ((g // CS) * CSP + g % CS).astype(np.int32)


def build_program(M1, M2):
    key = (M1, M2)
    if key in _cache:
        return _cache[key]
    nc = bacc.Bacc("TRN2", target_bir_lowering=False, debug=False,
                   num_devices=NCORES)
    Mmax = max(M1, M2)
    OFF, NW = _blob_layout(M1, M2)

    blob = nc.dram_tensor("blob", [NW], F32, kind="ExternalInput")
    out = nc.dram_tensor("out", [DECN, 1], F32, kind="ExternalOutput")

    def bl2d(name, rows, cols, dt=F32):
        """2-D [rows, cols] view of a blob segment (row-major)."""
        ap = blob[OFF[name]:OFF[name] + rows * cols].rearrange(
            "(p k) -> p k", k=cols)
        return ap if dt is F32 else ap.bitcast(dt)

    with tile.TileContext(nc) as tc, ExitStack() as ctx:
        dram = ctx.enter_context(tc.tile_pool(name="dram", bufs=1, space="DRAM"))
        cst = ctx.enter_context(tc.tile_pool(name="cst", bufs=1))
        res = ctx.enter_context(tc.tile_pool(name="res", bufs=1))
        sb = ctx.enter_context(tc.tile_pool(name="sb", bufs=2))
        msgp = ctx.enter_context(tc.tile_pool(name="msgp", bufs=2))
        ps = ctx.enter_context(tc.tile_pool(name="ps", bufs=2, space="PSUM"))

        # DRAM intermediates
        p_shard = dram.tile([PSP, HID], F32)
        px_shard = dram.tile([PSP, HID], F32)
        zp_shard = dram.tile([PSP, OUT], F32)
        zc_shard = dram.tile([CSP, OUT], F32)
        pT_dram = dram.tile([P, PSP], F32)    # p.T (item layer1, self path)
        cxT_dram = dram.tile([P, CSP], F32)   # cx.T (user cust layer1)
        p_full = dram.tile([PFULL, HID], F32, addr_space="Shared")
        px_full = dram.tile([PFULL, HID], F32, addr_space="Shared")
        zp_full = dram.tile([PFULL, OUT], F32, addr_space="Shared")
        zc_full = dram.tile([CFULL, OUT], F32, addr_space="Shared")
        x_sh_int = dram.tile([PSP, HID], F32)
        x_full = dram.tile([PFULL, HID], F32, addr_space="Shared")

        # constants
        ident = cst.tile([P, P], F32)
        make_identity(nc, ident[:])
        iota_i = cst.tile([P, Mmax * P], I32)
        nc.gpsimd.iota(iota_i[:].rearrange("p (m f) -> p m f", f=P),
                       pattern=[[0, Mmax], [1, P]], base=0, channel_multiplier=0)
        iota_f = cst.tile([P, Mmax * P], F32)
        nc.vector.tensor_copy(out=iota_f[:], in_=iota_i[:])

        def load_const(name, rows, cols, dt=F32, tag=None):
            dst = cst.tile([rows, cols], dt, tag=tag or name)
            nc.sync.dma_start(out=dst[:], in_=bl2d(name, rows, cols, dt))
            return dst

        # wcat DRAM layout: 10 consecutive [128,128] row-major blocks
        wcat_t = cst.tile([P, 10 * HID], F32, tag="wcat")
        nc.sync.dma_start(
            out=wcat_t[:].rearrange("p (w k) -> p w k", w=10),
            in_=blob[OFF["wcat"]:OFF["wcat"] + 10 * HID * HID].rearrange(
                "(w p k) -> p w k", w=10, k=HID))
        wnames = ["itW1lT", "itW1rT", "usW1lT", "usW1rT", "itW2lT", "itW2rT",
                  "usW2lT", "usW2rT", "usW3lT", "usW3rT"]
        w_t = {n: wcat_t[:, i * HID:(i + 1) * HID] for i, n in enumerate(wnames)}
        WpT_t = load_const("WpT", HID, OUT)
        WcT_t = load_const("WcT", HID, OUT)
        bias8_t = load_const("bias8", P, 8)
        b_t = {n: bias8_t[:, i:i + 1] for i, n in
               enumerate(["itb1", "usb1", "itb2", "usb2", "usb3"])}
        bc_ap = bias8_t[0:OUT, 5:6]
        b2_ap = bias8_t[:, 6:7]
        w2_t = load_const("w2rep", P, OUT)
        idxpp_t = load_const("idx_pp", P, PT * M1, I32)
        idxpc_t = load_const("idx_pc", P, CT * M2, I32)
        decc_t = load_const("dec_col", P, DEC, I32)

        def load_narrow(name, cols, ndt, odt, tag_sb):
            """u8/u16-packed blob segment -> converted cst tile [P, cols]."""
            words = 4 // np.dtype(mybir.dt.np(ndt)).itemsize
            src = blob[OFF[name]:OFF[name] + P * cols // words].bitcast(
                ndt).rearrange("(p k) -> p k", k=cols)
            tn = sb.tile([P, cols], ndt, tag=tag_sb)
            nc.sync.dma_start(out=tn[:], in_=src)
            tf = cst.tile([P, cols], odt, tag=name)
            nc.vector.tensor_copy(out=tf[:], in_=tn[:])
            return tf

        doffpp_t = load_narrow("doff_pp", PT * M1, mybir.dt.uint8, F32, "u8")
        doffpc_t = load_narrow("doff_pc", CT * M2, mybir.dt.uint8, F32, "u8")
        decr_t = load_narrow("dec_row", DEC, mybir.dt.uint16, I32, "u16")

        # residents: transposed input features (built below on PE)
        xpT_res = res.tile([P, PSP], F32)
        xcT_res = res.tile([P, CSP], F32)

        def loop_tiles(n, body):
            if USE_FORI and n > 4:
                with tc.For_i(0, n) as i:
                    body(i)
            else:
                for i in range(n):
                    body(i)

        # ---- ingest x shards: stage for AllGather + transpose for self path
        def x_ingest(name, ntiles, resid, stage_dram):
            def body(t):
                src = blob[ds(OFF[name] + t * (P * HID), P * HID)].rearrange(
                    "(p k) -> p k", k=HID)
                x32 = sb.tile([P, HID], F32, tag="x32")
                nc.sync.dma_start(out=x32[:], in_=src)
                if stage_dram is not None:
                    nc.sync.dma_start(out=stage_dram[ds(t * P, P), :],
                                      in_=x32[:])
                tp = ps.tile([P, P], F32, tag="pmT", space="PSUM")
                nc.tensor.transpose(out=tp[:], in_=x32[:], identity=ident[:])
                nc.vector.tensor_copy(out=resid[:, ds(t * P, P)], in_=tp[:])
            loop_tiles(ntiles, body)

        x_ingest("xp32", PT, xpT_res[:], x_sh_int)
        x_ingest("xc32", CT, xcT_res[:], None)

        def sage_pass(ntiles, M, idx_t, doff_t, table_ap, self_rhs, branches):
            """branches: list of (WlT_ap, WrT_ap, bias_ap, sink(t, pl_psum))"""
            def body(t):
                msg = msgp.tile([P, M * 129], F32, tag="msg")
                msg3 = msg[:].rearrange("p (m f) -> p m f", f=129)
                nc.vector.memset(msg3[:, :, 128:129], 1.0)
                # indirect offsets must live at a static address: stage the
                # tile's index columns into a fixed slot first
                ixs = sb.tile([P, M], I32, tag="ixs")
                nc.vector.tensor_copy(out=ixs[:], in_=idx_t[:, ds(t * M, M)])
                for m in range(M):
                    nc.gpsimd.indirect_dma_start(
                        out=msg3[:, m, 0:128], out_offset=None, in_=table_ap,
                        in_offset=bass.IndirectOffsetOnAxis(
                            ap=ixs[:, m:m + 1], axis=0))
                oh = msgp.tile([P, M * P], F32, tag="oh")
                nc.vector.tensor_tensor(
                    out=oh[:].rearrange("p (m f) -> p m f", f=P),
                    in0=doff_t[:, ds(t * M, M), None].to_broadcast([P, M, P]),
                    in1=iota_f[:, 0:M * P].rearrange("p (m f) -> p m f", f=P),
                    op=mybir.AluOpType.is_equal)
                pagg = ps.tile([P, 129], F32, tag="pagg", space="PSUM")
                for m in range(M):
                    nc.tensor.matmul(out=pagg[:], lhsT=oh[:, m * P:(m + 1) * P],
                                     rhs=msg3[:, m, :], start=(m == 0),
                                     stop=(m == M - 1))
                inv = sb.tile([P, 1], F32, tag="inv")
                nc.vector.tensor_scalar_max(out=inv[:], in0=pagg[:, 128:129],
                                            scalar1=1.0)
                nc.vector.reciprocal(out=inv[:], in_=inv[:])
                mean = sb.tile([P, P], F32, tag="mean")
                nc.vector.tensor_scalar_mul(out=mean[:], in0=pagg[:, 0:128],
                                            scalar1=inv[:, 0:1])
                mT_ps = ps.tile([P, P], F32, tag="pmT", space="PSUM")
                nc.tensor.transpose(out=mT_ps[:], in_=mean[:], identity=ident[:])
                mT = sb.tile([P, P], F32, tag="mT")
                nc.vector.tensor_copy(out=mT[:], in_=mT_ps[:])
                xT = self_rhs(t)
                for WlT_ap, WrT_ap, bias_ap, sink in branches:
                    pl = ps.tile([P, P], F32, tag="plin", space="PSUM")
                    nc.tensor.matmul(out=pl[:], lhsT=WlT_ap, rhs=mT[:],
                                     start=True, stop=False)
                    nc.tensor.matmul(out=pl[:], lhsT=WrT_ap, rhs=xT,
                                     start=False, stop=True)
                    sink(t, pl, bias_ap)
            loop_tiles(ntiles, body)

        def sink_store(featT_dram, nodemajor_dram):
            """relu -> optionally [feat,dst] DRAM slab and/or transposed
            node-major DRAM (for AllGather)."""
            def f(t, pl, bias_ap):
                ht = sb.tile([P, P], F32, tag="h")
                nc.scalar.activation(out=ht[:], in_=pl[:],
                                     func=mybir.ActivationFunctionType.Relu,
                                     bias=bias_ap)
                if featT_dram is not None:
                    nc.sync.dma_start(out=featT_dram[:, ds(t * P, P)],
                                      in_=ht[:])
                if nodemajor_dram is not None:
                    tp = ps.tile([P, P], F32, tag="ptr", space="PSUM")
                    nc.tensor.transpose(out=tp[:], in_=ht[:], identity=ident[:])
                    hT = sb.tile([P, P], F32, tag="hT")
                    nc.vector.tensor_copy(out=hT[:], in_=tp[:])
                    nc.sync.dma_start(out=nodemajor_dram[ds(t * P, P), :],
                                      in_=hT[:])
            return f

        def sink_z(WzT_ap, bz_ap, z_dram):
            """h2 = relu(pl); z = WzT.T @ h2 (+bz); transpose; DMA [d, OUT]"""
            def f(t, pl, bias_ap):
                ht = sb.tile([P, P], F32, tag="h")
                nc.scalar.activation(out=ht[:], in_=pl[:],
                                     func=mybir.ActivationFunctionType.Relu,
                                     bias=bias_ap)
                pz = ps.tile([OUT, P], F32, tag="plin", space="PSUM")
                nc.tensor.matmul(out=pz[:], lhsT=WzT_ap, rhs=ht[:],
                                 start=True, stop=True)
                zsb = sb.tile([OUT, P], F32, tag="zsb")
                if bz_ap is not None:
                    nc.vector.tensor_scalar_add(out=zsb[:], in0=pz[:],
                                                scalar1=bz_ap)
                else:
                    nc.vector.tensor_copy(out=zsb[:], in_=pz[:])
                tp = ps.tile([P, OUT], F32, tag="ptr", space="PSUM")
                nc.tensor.transpose(out=tp[:], in_=zsb[:],
                                    identity=ident[0:OUT, 0:OUT])
                zT = sb.tile([P, OUT], F32, tag="hT")
                nc.vector.tensor_copy(out=zT[:], in_=tp[:])
                nc.sync.dma_start(out=z_dram[ds(t * P, P), :], in_=zT[:])
            return f

        def stream_selfT(src_dram):
            def f(t):
                xt = sb.tile([P, P], F32, tag="xT")
                nc.sync.dma_start(out=xt[:], in_=src_dram[:, ds(t * P, P)])
                return xt[:]
            return f

        # ---- AllGather x_product shards -> x_full
        rg = [list(range(NCORES))]
        nc.gpsimd.collective_compute("AllGather", mybir.AluOpType.bypass,
                                     replica_groups=rg, ins=[x_sh_int.opt()],
                                     outs=[x_full.opt()])

        # ---- pass A1: pp edges -> mean1 -> p (item) & px (user), shared agg
        sage_pass(PT, M1, idxpp_t[:], doffpp_t[:], x_full[:],
                  lambda t: xpT_res[:, ds(t * P, P)],
                  [(w_t["itW1lT"], w_t["itW1rT"], b_t["itb1"],
                    sink_store(pT_dram, p_shard)),
                   (w_t["usW1lT"], w_t["usW1rT"], b_t["usb1"],
                    sink_store(None, px_shard))])

        # ---- pass B1: pc edges (x_prod -> cust) -> cx
        sage_pass(CT, M2, idxpc_t[:], doffpc_t[:], x_full[:],
                  lambda t: xcT_res[:, ds(t * P, P)],
                  [(w_t["usW2lT"], w_t["usW2rT"], b_t["usb2"],
                    sink_store(cxT_dram, None))])

        # ---- AllGather p, px
        nc.gpsimd.collective_compute("AllGather", mybir.AluOpType.bypass,
                                     replica_groups=rg, ins=[p_shard.opt()],
                                     outs=[p_full.opt()])
        nc.gpsimd.collective_compute("AllGather", mybir.AluOpType.bypass,
                                     replica_groups=rg, ins=[px_shard.opt()],
                                     outs=[px_full.opt()])

        # ---- pass A2: pp edges over p -> p2 -> ZP'
        sage_pass(PT, M1, idxpp_t[:], doffpp_t[:], p_full[:],
                  stream_selfT(pT_dram),
                  [(w_t["itW2lT"], w_t["itW2rT"], b_t["itb2"],
                    sink_z(WpT_t[:], None, zp_shard))])

        # ---- pass B2: pc edges over px -> cx2 -> ZC'
        sage_pass(CT, M2, idxpc_t[:], doffpc_t[:], px_full[:],
                  stream_selfT(cxT_dram),
                  [(w_t["usW3lT"], w_t["usW3rT"], b_t["usb3"],
                    sink_z(WcT_t[:], bc_ap, zc_shard))])

        # ---- AllGather ZP', ZC'
        nc.gpsimd.collective_compute("AllGather", mybir.AluOpType.bypass,
                                     replica_groups=rg, ins=[zp_shard.opt()],
                                     outs=[zp_full.opt()])
        nc.gpsimd.collective_compute("AllGather", mybir.AluOpType.bypass,
                                     replica_groups=rg, ins=[zc_shard.opt()],
                                     outs=[zc_full.opt()])

        # ---- decoder
        acc = res.tile([P, DEC], F32)

        def dec_body(g, w):
            zcq = sb.tile([P, GD * OUT], F32, tag="zcq")
            zpq = sb.tile([P, GD * OUT], F32, tag="zpq")
            dst = sb.tile([P, 2 * GD], I32, tag="dst")
            nc.vector.tensor_copy(out=dst[:, 0:w], in_=decr_t[:, ds(g * GD, w)])
            nc.vector.tensor_copy(out=dst[:, GD:GD + w],
                                  in_=decc_t[:, ds(g * GD, w)])
            for j in range(w):
                nc.gpsimd.indirect_dma_start(
                    out=zcq[:, j * OUT:(j + 1) * OUT], out_offset=None,
                    in_=zc_full[:],
                    in_offset=bass.IndirectOffsetOnAxis(
                        ap=dst[:, j:j + 1], axis=0))
                nc.gpsimd.indirect_dma_start(
                    out=zpq[:, j * OUT:(j + 1) * OUT], out_offset=None,
                    in_=zp_full[:],
                    in_offset=bass.IndirectOffsetOnAxis(
                        ap=dst[:, GD + j:GD + j + 1], axis=0))
            sq = sb.tile([P, GD * OUT], F32, tag="sq")
            nc.vector.tensor_tensor(out=sq[:, 0:w * OUT], in0=zcq[:, 0:w * OUT],
                                    in1=zpq[:, 0:w * OUT],
                                    op=mybir.AluOpType.add)
            rq = sb.tile([P, GD * OUT], F32, tag="rq")
            nc.scalar.activation(out=rq[:, 0:w * OUT], in_=sq[:, 0:w * OUT],
                                 func=mybir.ActivationFunctionType.Relu)
            mq = sb.tile([P, GD * OUT], F32, tag="mq")
            nc.vector.tensor_tensor(
                out=mq[:].rearrange("p (j f) -> p j f", f=OUT)[:, 0:w, :],
                in0=rq[:].rearrange("p (j f) -> p j f", f=OUT)[:, 0:w, :],
                in1=w2_t[:, None, 0:OUT].to_broadcast([P, w, OUT]),
                op=mybir.AluOpType.mult)
            nc.vector.reduce_sum(
                out=acc[:, ds(g * GD, w)],
                in_=mq[:].rearrange("p (j f) -> p j f", f=OUT)[:, 0:w, :],
                axis=mybir.AxisListType.X)

        ngf = DEC // GD                       # full groups
        loop_tiles(ngf, lambda g: dec_body(g, GD))
        if DEC - ngf * GD:
            dec_body(ngf, DEC - ngf * GD)     # static tail
        acc_b = res.tile([P, DEC], F32)
        nc.vector.tensor_scalar_add(out=acc_b[:], in0=acc[:], scalar1=b2_ap)
        outv = out[:, :].rearrange("(c p) o -> c (p o)", p=P)
        for b in range((DEC + P - 1) // P):
            w = min(P, DEC - b * P)
            tp = ps.tile([P, P], F32, tag="ptr", space="PSUM")
            nc.tensor.transpose(out=tp[0:w, :], in_=acc_b[:, b * P:b * P + w],
                                identity=ident[:])
            ts = sb.tile([P, P], F32, tag="hT")
            nc.vector.tensor_copy(out=ts[0:w, :], in_=tp[0:w, :])
            nc.sync.dma_start(out=outv[b * P:b * P + w, :], in_=ts[0:w, :])

    nc.compile()
    _cache[(M1, M2)] = nc
    return nc


def kernel(**inputs):
    x_product = np.ascontiguousarray(np.asarray(inputs["x_product"], np.float32))
    x_customer = np.ascontiguousarray(np.asarray(inputs["x_customer"], np.float32))
    ei_pp = np.asarray(inputs["ei_pp"])
    ei_pc = np.asarray(inputs["ei_pc"])
    eli = np.asarray(inputs["edge_label_index"])

    # host prep: edge bucketing (sharding) + weight folding
    bpp = _bucket_edges(ei_pp[0], ei_pp[1], PS, PT)
    bpc = _bucket_edges(ei_pc[0], ei_pc[1], CS, CT)
    M1, idx_pp, doff_pp = _edge_tiles(bpp, PT)
    M2, idx_pc, doff_pc = _edge_tiles(bpc, CT)
    idx_pp = [_remap_prod(a.astype(np.int64)) for a in idx_pp]
    idx_pc = [_remap_prod(a.astype(np.int64)) for a in idx_pc]

    row, col = eli[0].astype(np.int64), eli[1].astype(np.int64)
    dec_rows, dec_cols = [], []
    for c in range(NCORES):
        r = np.zeros(DECN, np.int64)
        q = np.zeros(DECN, np.int64)
        r[:EC] = row[c * EC:(c + 1) * EC]
        q[:EC] = col[c * EC:(c + 1) * EC]
        dec_rows.append(np.ascontiguousarray(
            _remap_cust(r).reshape(DEC, P).T).astype(np.uint16))
        dec_cols.append(np.ascontiguousarray(
            _remap_prod(q).reshape(DEC, P).T))

    f32 = lambda a: np.ascontiguousarray(np.asarray(a, np.float32))
    W = {k: f32(inputs[k]) for k in
         ["it_W1l", "it_W1r", "it_W2l", "it_W2r", "it_Wlin",
          "us_W1l", "us_W1r", "us_W2l", "us_W2r", "us_W3l", "us_W3r",
          "us_Wlin", "de_W1", "de_W2"]}
    b = {k: f32(inputs[k]) for k in
         ["it_b1", "it_b2", "it_blin", "us_b1", "us_b2", "us_b3", "us_blin",
          "de_b1", "de_b2"]}
    W1L, W1R = W["de_W1"][:, :OUT], W["de_W1"][:, OUT:]

    wcat = np.concatenate([
        W["it_W1l"].T, W["it_W1r"].T, W["us_W1l"].T, W["us_W1r"].T,
        W["it_W2l"].T, W["it_W2r"].T, W["us_W2l"].T, W["us_W2r"].T,
        W["us_W3l"].T, W["us_W3r"].T], axis=0)
    bias8 = np.zeros((P, 8), np.float32)
    for i, k in enumerate(["it_b1", "us_b1", "it_b2", "us_b2", "us_b3"]):
        bias8[:, i] = b[k]
    bias8[:OUT, 5] = b["de_b1"] + W1L @ b["us_blin"] + W1R @ b["it_blin"]
    bias8[:, 6] = np.float32(b["de_b2"].reshape(-1)[0])
    w2rep = np.tile(W["de_W2"].reshape(1, OUT), (P, 1))

    shared_f32 = [f32(wcat).ravel(), f32((W1R @ W["it_Wlin"]).T).ravel(),
                  f32((W1L @ W["us_Wlin"]).T).ravel(), bias8.ravel(),
                  f32(w2rep).ravel()]

    xpad = np.zeros((PSP, HID), np.float32)
    cpad = np.zeros((CSP, HID), np.float32)
    in_maps = []
    for c in range(NCORES):
        xpad[:PS] = x_product[c * PS:(c + 1) * PS]
        cpad[:CS] = x_customer[c * CS:(c + 1) * CS]
        parts = ([xpad.ravel().copy(), cpad.ravel().copy(),
                  doff_pp[c].ravel().view(np.float32),
                  doff_pc[c].ravel().view(np.float32)]
                 + shared_f32
                 + [idx_pp[c].ravel().view(np.float32),
                    idx_pc[c].ravel().view(np.float32),
                    dec_rows[c].ravel().view(np.float32),
                    dec_cols[c].ravel().view(np.float32)])
        in_maps.append({"blob": np.concatenate(parts)})

    nc = build_program(M1, M2)
    res = bass_utils.run_bass_kernel_spmd(nc, in_maps,
                                          core_ids=list(range(NCORES)))
    kernel.last_in_maps = in_maps
    kernel.last_nc = nc
    return np.concatenate([res.results[c]["out"][:EC] for c in range(NCORES)],
                          axis=0).astype(np.float32)
